# Initial kernel scaffold
#
"""Trainium2 Bass kernel for nn_CausalFFNN (pairwise relu-MLP scores).

Computes: Hn = relu(relu(E@W1+b1)@W2+b2)
          logits[i,j] = relu(Hn[i]@Wa + Hn[j]@Wb + bp1) @ Wp2 + bp2
          out = softplus(logits), diag = 0
Sharding: i-rows split across 8 cores (128 rows each); weights + full E
replicated. Each core computes a (128, 1024) output slab.
"""
import sys
import os
import tempfile
import numpy as np

os.environ["NEURON_COMPILE_CACHE_URL"] = tempfile.mkdtemp(prefix="neuron-cache-")

for _p in ("/opt/trn_rl_repo", "/root/.axon_site/_ro/trn_rl_repo"):
    if os.path.isdir(_p) and _p not in sys.path:
        sys.path.insert(0, _p)

N, D, HID = 1024, 512, 256
NCORE = 8
SLAB = N // NCORE          # 128 i-rows per core
P = 128
G = 8                      # i's per PSUM col strip
HSUB = HID // 16           # 16 h-components per chunk per i
NT = HID // HSUB           # 16 chunks
NR = 4                     # rounds (free-dim offsets in PSUM)
NS = 4                     # col strips
NJT = 2                    # 512-wide j tiles per 1024

_CACHE = {}


def _build_program(repeat=1):
    import concourse.bacc as bacc
    import concourse.mybir as mybir
    from concourse.tile import TileContext

    F32 = mybir.dt.float32
    F32R = mybir.dt.float32r
    F16 = mybir.dt.float16
    AF = mybir.ActivationFunctionType
    ALU = mybir.AluOpType

    nc = bacc.Bacc("TRN2", target_bir_lowering=False, debug=False)

    # ---- DRAM parameters (per core; arrays supplied via in_maps) ----
    dET = nc.dram_tensor("ET", [D, N], F32R, kind="ExternalInput")
    dETs = nc.dram_tensor("ETs", [D, SLAB], F32R, kind="ExternalInput")
    dW1s = nc.dram_tensor("W1s", [P, 4 * HID], F32R, kind="ExternalInput")
    dW2s = nc.dram_tensor("W2s", [P, 2 * HID], F32R, kind="ExternalInput")
    dWas = nc.dram_tensor("Was", [P, 2 * HID], F32R, kind="ExternalInput")
    dWbrep = nc.dram_tensor("Wbrep", [NT, 2, P, P], F32R, kind="ExternalInput")
    db1 = nc.dram_tensor("b1c", [P, 2], F32, kind="ExternalInput")
    db2 = nc.dram_tensor("b2c", [P, 2], F32, kind="ExternalInput")
    dbp1 = nc.dram_tensor("bp1c", [P, 2], F32, kind="ExternalInput")
    dbp2 = nc.dram_tensor("bp2c", [P, 1], F32, kind="ExternalInput")
    dWst = nc.dram_tensor("Wst", [P, NT * G], F16, kind="ExternalInput")
    dY = nc.dram_tensor("Y", [SLAB, N], F32, kind="ExternalOutput")

    with TileContext(nc) as tc:
        with tc.tile_pool(name="const", bufs=1) as cpool, \
             tc.tile_pool(name="work", bufs=1) as wpool, \
             tc.tile_pool(name="rpool", bufs=12) as rpool, \
             tc.tile_pool(name="epool", bufs=3) as epool, \
             tc.tile_pool(name="dpool", bufs=1, space="DRAM") as dpool:

            # ---------- load constants (small/urgent first) ----------
            W1s = cpool.tile([P, 4 * HID], F32R, tag="W1s")
            nc.sync.dma_start(W1s[:], dW1s.ap())
            W2s = cpool.tile([P, 2 * HID], F32R, tag="W2s")
            nc.sync.dma_start(W2s[:], dW2s.ap())
            Was = cpool.tile([P, 2 * HID], F32R, tag="Was")
            nc.sync.dma_start(Was[:], dWas.ap())
            b1c = cpool.tile([P, 2], F32, tag="b1c")
            nc.sync.dma_start(b1c[:], db1.ap())
            b2c = cpool.tile([P, 2], F32, tag="b2c")
            nc.sync.dma_start(b2c[:], db2.ap())
            bp1c = cpool.tile([P, 2], F32, tag="bp1c")
            nc.sync.dma_start(bp1c[:], dbp1.ap())
            bp2c = cpool.tile([P, 1], F32, tag="bp2c")
            nc.sync.dma_start(bp2c[:], dbp2.ap())
            Wst = cpool.tile([P, NT * G], F16, tag="Wst")
            nc.sync.dma_start(Wst[:], dWst.ap())
            ETs = cpool.tile([P, 4 * SLAB], F32R, tag="ETs")
            for kd in range(4):
                nc.sync.dma_start(ETs[:, kd * SLAB:(kd + 1) * SLAB],
                                  dETs.ap()[kd * P:(kd + 1) * P, :])
            ET = cpool.tile([P, 4 * N], F32R, tag="ET")
            for kd in range(4):
                nc.sync.dma_start(ET[:, kd * N:(kd + 1) * N],
                                  dET.ap()[kd * P:(kd + 1) * P, :])
            Wbrep = cpool.tile([P, NT * 2 * P], F32R, tag="Wbrep")
            nc.sync.dma_start(
                Wbrep[:].rearrange("p (t kh m) -> p t kh m", kh=2, m=P),
                dWbrep.ap().rearrange("t kh p m -> p t kh m"))

            ATd = dpool.tile([HID, SLAB], F32, tag="ATd")
            H1T = wpool.tile([P, 2 * N], F32R, tag="H1T")
            HnT = wpool.tile([P, 2 * N], F32R, tag="HnT")
            H1Ts = wpool.tile([P, 2 * SLAB], F32R, tag="H1Ts")
            HnTs = wpool.tile([P, 2 * SLAB], F32R, tag="HnTs")
            ATs = wpool.tile([P, 2 * SLAB], F32, tag="ATs")
            CTS = wpool.tile([P, NT * N], F16, tag="CTS")
            BT = wpool.tile([P, NT * 16], F32, tag="BT")

            def compute_body():
                # ---------- encoder (transposed, f32r) ----------
                with tc.tile_pool(name="eps", bufs=4, space="PSUM") as pps:
                    # H1T = relu(W1^T @ E^T + b1)
                    for mh in range(2):
                        for jt in range(2):
                            ps = pps.tile([P, 512], F32, tag="eps")
                            for kd in range(4):
                                nc.tensor.matmul(
                                    ps[:],
                                    W1s[:, kd * HID + mh * P: kd * HID + (mh + 1) * P],
                                    ET[:, kd * N + jt * 512: kd * N + (jt + 1) * 512],
                                    start=(kd == 0), stop=(kd == 3))
                            dstv = H1T[:, mh * N + jt * 512: mh * N + (jt + 1) * 512]
                            if jt == 0:
                                nc.scalar.activation(dstv, ps[:], AF.Relu,
                                                     bias=b1c[:, mh:mh + 1])
                            else:
                                nc.vector.tensor_scalar(dstv, ps[:], b1c[:, mh:mh + 1],
                                                        0.0, ALU.add, ALU.max)
                    # HnT = relu(W2^T @ H1T + b2)
                    for mh in range(2):
                        for jt in range(2):
                            ps = pps.tile([P, 512], F32, tag="eps")
                            for kh in range(2):
                                nc.tensor.matmul(
                                    ps[:],
                                    W2s[:, kh * HID + mh * P: kh * HID + (mh + 1) * P],
                                    H1T[:, kh * N + jt * 512: kh * N + (jt + 1) * 512],
                                    start=(kh == 0), stop=(kh == 1))
                            dstv = HnT[:, mh * N + jt * 512: mh * N + (jt + 1) * 512]
                            if jt == 0:
                                nc.scalar.activation(dstv, ps[:], AF.Relu,
                                                     bias=b2c[:, mh:mh + 1])
                            else:
                                nc.vector.tensor_scalar(dstv, ps[:], b2c[:, mh:mh + 1],
                                                        0.0, ALU.add, ALU.max)
                    # slab encoder: H1Ts, HnTs, ATs (Nf = 128)
                    for mh in range(2):
                        ps = pps.tile([P, SLAB], F32, tag="sps")
                        for kd in range(4):
                            nc.tensor.matmul(
                                ps[:],
                                W1s[:, kd * HID + mh * P: kd * HID + (mh + 1) * P],
                                ETs[:, kd * SLAB:(kd + 1) * SLAB],
                                start=(kd == 0), stop=(kd == 3))
                        nc.scalar.activation(
                            H1Ts[:, mh * SLAB:(mh + 1) * SLAB],
                            ps[:], AF.Relu, bias=b1c[:, mh:mh + 1])
                    for mh in range(2):
                        ps = pps.tile([P, SLAB], F32, tag="sps")
                        for kh in range(2):
                            nc.tensor.matmul(
                                ps[:],
                                W2s[:, kh * HID + mh * P: kh * HID + (mh + 1) * P],
                                H1Ts[:, kh * SLAB:(kh + 1) * SLAB],
                                start=(kh == 0), stop=(kh == 1))
                        nc.scalar.activation(
                            HnTs[:, mh * SLAB:(mh + 1) * SLAB],
                            ps[:], AF.Relu, bias=b2c[:, mh:mh + 1])
                    for mh in range(2):
                        ps = pps.tile([P, SLAB], F32, tag="sps")
                        for kh in range(2):
                            nc.tensor.matmul(
                                ps[:],
                                Was[:, kh * HID + mh * P: kh * HID + (mh + 1) * P],
                                HnTs[:, kh * SLAB:(kh + 1) * SLAB],
                                start=(kh == 0), stop=(kh == 1))
                        nc.scalar.activation(
                            ATs[:, mh * SLAB:(mh + 1) * SLAB],
                            ps[:], AF.Identity, bias=bp1c[:, mh:mh + 1])

                # ---------- BT via DRAM bounce ----------
                for mh in range(2):
                    nc.sync.dma_start(ATd[mh * P:(mh + 1) * P, :],
                                      ATs[:, mh * SLAB:(mh + 1) * SLAB])
                # group c = r*4+s owns i's {g*16+c}; host un-permutes rows
                atd_v = ATd[:].rearrange("(t u) (gg c) -> gg u t c",
                                           u=HSUB, gg=G)
                for g in range(G):
                    dst = BT[g * HSUB:(g + 1) * HSUB, :] \
                        .rearrange("u (t c) -> u t c", c=16)
                    nc.sync.dma_start(dst, atd_v[g])

                # ---------- CTS via replicated-Wb matmuls ----------
                with tc.tile_pool(name="cps", bufs=4, space="PSUM") as cpps:
                    for t in range(NT):
                        for jt in range(2):
                            ps = cpps.tile([P, 512], F32, tag="cps")
                            for kh in range(2):
                                nc.tensor.matmul(
                                    ps[:],
                                    Wbrep[:, (t * 2 + kh) * P:(t * 2 + kh + 1) * P],
                                    HnT[:, kh * N + jt * 512: kh * N + (jt + 1) * 512],
                                    start=(kh == 0), stop=(kh == 1))
                            dst = CTS[:, t * N + jt * 512: t * N + (jt + 1) * 512]
                            if (t * 2 + jt) % 2 == 0:
                                nc.scalar.copy(dst, ps[:])
                            else:
                                nc.vector.tensor_copy(dst, ps[:])

                # ---------- pairwise main loop ----------
                with tc.tile_pool(name="mps", bufs=1, space="PSUM") as mpool:
                    PS = mpool.tile([P, NR * N], F32, tag="PS")
                    ESB = epool.tile([P, N], F32, tag="ESB")
                    OUTSB = wpool.tile([P, N], F32, tag="OUTSB")
                    OUT2 = wpool.tile([P, N], F32, tag="OUT2")
                    for r in range(NR):
                        for t in range(NT):
                            for s in range(NS):
                                R = rpool.tile([P, N], F16, tag="R")
                                bias_col = BT[:, t * 16 + r * 4 + s: t * 16 + r * 4 + s + 1]
                                src = CTS[:, t * N:(t + 1) * N]
                                rel_idx = (r * NT + t) * NS + s
                                if rel_idx % 6 == 0:
                                    nc.scalar.activation(R[:], src, AF.Relu,
                                                         bias=bias_col)
                                else:
                                    nc.vector.tensor_scalar(R[:], src, bias_col,
                                                            0.0, ALU.add, ALU.max)
                                for jt in range(NJT):
                                    nc.tensor.matmul(
                                        PS[32 * s:32 * s + G,
                                           r * N + jt * 512: r * N + (jt + 1) * 512],
                                        Wst[:, t * G:(t + 1) * G],
                                        R[:, jt * 512:(jt + 1) * 512],
                                        start=(t == 0), stop=(t == NT - 1),
                                        tile_position=(0, 32 * s))
                        # drain round r: exp(logits + bp2) from PSUM, compact via DMA
                        for s in range(NS):
                            nc.scalar.activation(
                                ESB[32 * s:32 * s + G, :],
                                PS[32 * s:32 * s + G, r * N:(r + 1) * N],
                                AF.Exp, bias=bp2c[32 * s:32 * s + G, 0:1])
                            nc.sync.dma_start(
                                OUTSB[r * 32 + s * G: r * 32 + (s + 1) * G, :],
                                ESB[32 * s:32 * s + G, :])
                    # softplus = ln(1 + exp(x)); diagonal fixed on host
                    OUT2 = wpool.tile([P, N], F32, tag="OUT2")
                    nc.scalar.activation(OUT2[:], OUTSB[:], AF.Ln, bias=1.0)
                    nc.sync.dma_start(dY.ap(), OUT2[:])

            if repeat == 1:
                compute_body()
            else:
                with tc.For_i(0, repeat, 1):
                    compute_body()

    nc.compile()
    return nc


def _prep_inputs(E, W1, b1, W2, b2, Wp1, bp1, Wp2, bp2):
    f32 = np.float32
    E = np.asarray(E, f32)
    W1 = np.asarray(W1, f32)
    b1 = np.asarray(b1, f32)
    W2 = np.asarray(W2, f32)
    b2 = np.asarray(b2, f32)
    Wp1 = np.asarray(Wp1, f32)
    bp1 = np.asarray(bp1, f32)
    Wp2 = np.asarray(Wp2, f32)
    bp2 = np.asarray(bp2, f32)

    ET = np.ascontiguousarray(E.T)                      # (512, 1024)
    W1s = np.ascontiguousarray(
        W1.reshape(4, P, HID).transpose(1, 0, 2).reshape(P, 4 * HID))
    W2s = np.ascontiguousarray(
        W2.reshape(2, P, HID).transpose(1, 0, 2).reshape(P, 2 * HID))
    Wa, Wb = Wp1[:HID], Wp1[HID:]
    Was = np.ascontiguousarray(
        Wa.reshape(2, P, HID).transpose(1, 0, 2).reshape(P, 2 * HID))
    Wbrep = np.zeros((NT, 2, P, P), np.float32)
    for t in range(NT):
        for kh in range(2):
            Wbrep[t, kh] = np.tile(Wb[kh * P:(kh + 1) * P, t * HSUB:(t + 1) * HSUB],
                                   (1, G))
    b1c = np.ascontiguousarray(b1.reshape(2, P).T)
    b2c = np.ascontiguousarray(b2.reshape(2, P).T)
    bp1c = np.ascontiguousarray(bp1.reshape(2, P).T)

    Wst = np.zeros((P, NT * G), np.float16)
    w = Wp2[:, 0]
    for t in range(NT):
        for g in range(G):
            for u in range(HSUB):
                Wst[g * HSUB + u, t * G + g] = w[t * HSUB + u]

    bp2c = np.full((P, 1), bp2[0], np.float32)
    common = {
        "ET": ET, "W1s": W1s, "W2s": W2s, "Was": Was, "Wbrep": Wbrep,
        "b1c": b1c, "b2c": b2c, "bp1c": bp1c, "bp2c": bp2c, "Wst": Wst,
    }
    in_maps = []
    for k in range(NCORE):
        m = dict(common)
        m["ETs"] = np.ascontiguousarray(E[k * SLAB:(k + 1) * SLAB, :].T)
        in_maps.append(m)
    return in_maps, float(bp2[0])


def kernel(E, W1, b1, W2, b2, Wp1, bp1, Wp2, bp2):
    from concourse.bass_utils import run_bass_kernel_spmd

    if "nc" not in _CACHE:
        _CACHE["nc"] = _build_program()
    nc = _CACHE["nc"]

    in_maps, _ = _prep_inputs(E, W1, b1, W2, b2, Wp1, bp1, Wp2, bp2)
    res = run_bass_kernel_spmd(nc, in_maps, list(range(NCORE)))
    # device writes row p = c*8+g for slab-local i = g*16+c; un-permute
    slabs = [res.results[k]["Y"].reshape(16, 8, N).transpose(1, 0, 2).reshape(SLAB, N)
             for k in range(NCORE)]
    out = np.concatenate(slabs, axis=0)
    np.fill_diagonal(out, 0.0)
    return np.ascontiguousarray(out.astype(np.float32))



# revision 1
# speedup vs baseline: 1.3926x; 1.3926x over previous
"""Trainium2 Bass kernel for nn_CausalFFNN (pairwise relu-MLP scores).

Computes: Hn = relu(relu(E@W1+b1)@W2+b2)
          logits[i,j] = relu(Hn[i]@Wa + Hn[j]@Wb + bp1) @ Wp2 + bp2
          out = softplus(logits), diag = 0
Sharding: i-rows split across 8 cores (128 rows each); weights + full E
replicated. Each core computes a (128, 1024) output slab.
"""
import sys
import os
import tempfile
import numpy as np

os.environ["NEURON_COMPILE_CACHE_URL"] = tempfile.mkdtemp(prefix="neuron-cache-")

for _p in ("/opt/trn_rl_repo", "/root/.axon_site/_ro/trn_rl_repo"):
    if os.path.isdir(_p) and _p not in sys.path:
        sys.path.insert(0, _p)

N, D, HID = 1024, 512, 256
NCORE = 8
SLAB = N // NCORE          # 128 i-rows per core
P = 128
G = 8                      # i's per PSUM col strip
HSUB = HID // 16           # 16 h-components per chunk per i
NT = HID // HSUB           # 16 chunks
NR = 4                     # rounds (free-dim offsets in PSUM)
NS = 4                     # col strips
NJT = 2                    # 512-wide j tiles per 1024

_CACHE = {}


def _build_program(repeat=1):
    import concourse.bacc as bacc
    import concourse.mybir as mybir
    from concourse.tile import TileContext

    F32 = mybir.dt.float32
    F32R = mybir.dt.float32r
    F16 = mybir.dt.float16
    AF = mybir.ActivationFunctionType
    ALU = mybir.AluOpType

    nc = bacc.Bacc("TRN2", target_bir_lowering=False, debug=False)

    # ---- DRAM parameters (per core; arrays supplied via in_maps) ----
    dET = nc.dram_tensor("ET", [D, N], F32R, kind="ExternalInput")
    dETs = nc.dram_tensor("ETs", [D, SLAB], F32R, kind="ExternalInput")
    dW1s = nc.dram_tensor("W1s", [P, 4 * HID], F32R, kind="ExternalInput")
    dW2s = nc.dram_tensor("W2s", [P, 2 * HID], F32R, kind="ExternalInput")
    dWas = nc.dram_tensor("Was", [P, 2 * HID], F32R, kind="ExternalInput")
    dWbrep = nc.dram_tensor("Wbrep", [NT, 2, P, P], F32R, kind="ExternalInput")
    db1 = nc.dram_tensor("b1c", [P, 2], F32, kind="ExternalInput")
    db2 = nc.dram_tensor("b2c", [P, 2], F32, kind="ExternalInput")
    dbp1 = nc.dram_tensor("bp1c", [P, 2], F32, kind="ExternalInput")
    dbp2 = nc.dram_tensor("bp2c", [P, 1], F32, kind="ExternalInput")
    dWst = nc.dram_tensor("Wst", [P, NT * G], F16, kind="ExternalInput")
    dY = nc.dram_tensor("Y", [SLAB, N], F32, kind="ExternalOutput")

    with TileContext(nc) as tc:
        with tc.tile_pool(name="const", bufs=1) as cpool, \
             tc.tile_pool(name="work", bufs=1) as wpool, \
             tc.tile_pool(name="rpool", bufs=12) as rpool, \
             tc.tile_pool(name="epool", bufs=3) as epool, \
             tc.tile_pool(name="dpool", bufs=1, space="DRAM") as dpool:

            # ---------- load constants (small/urgent first) ----------
            W1s = cpool.tile([P, 4 * HID], F32R, tag="W1s")
            nc.sync.dma_start(W1s[:], dW1s.ap())
            W2s = cpool.tile([P, 2 * HID], F32R, tag="W2s")
            nc.sync.dma_start(W2s[:], dW2s.ap())
            Was = cpool.tile([P, 2 * HID], F32R, tag="Was")
            nc.sync.dma_start(Was[:], dWas.ap())
            b1c = cpool.tile([P, 2], F32, tag="b1c")
            nc.sync.dma_start(b1c[:], db1.ap())
            b2c = cpool.tile([P, 2], F32, tag="b2c")
            nc.sync.dma_start(b2c[:], db2.ap())
            bp1c = cpool.tile([P, 2], F32, tag="bp1c")
            nc.sync.dma_start(bp1c[:], dbp1.ap())
            bp2c = cpool.tile([P, 1], F32, tag="bp2c")
            nc.sync.dma_start(bp2c[:], dbp2.ap())
            Wst = cpool.tile([P, NT * G], F16, tag="Wst")
            nc.sync.dma_start(Wst[:], dWst.ap())
            ETs = cpool.tile([P, 4 * SLAB], F32R, tag="ETs")
            for kd in range(4):
                nc.sync.dma_start(ETs[:, kd * SLAB:(kd + 1) * SLAB],
                                  dETs.ap()[kd * P:(kd + 1) * P, :])
            ET = cpool.tile([P, 4 * N], F32R, tag="ET")
            for kd in range(4):
                nc.sync.dma_start(ET[:, kd * N:(kd + 1) * N],
                                  dET.ap()[kd * P:(kd + 1) * P, :])
            Wbrep = cpool.tile([P, NT * 2 * P], F32R, tag="Wbrep")
            nc.sync.dma_start(
                Wbrep[:].rearrange("p (t kh m) -> p t kh m", kh=2, m=P),
                dWbrep.ap().rearrange("t kh p m -> p t kh m"))

            ATd = dpool.tile([HID, SLAB], F32, tag="ATd")
            H1T = wpool.tile([P, 2 * N], F32R, tag="H1T")
            HnT = wpool.tile([P, 2 * N], F32R, tag="HnT")
            H1Ts = wpool.tile([P, 2 * SLAB], F32R, tag="H1Ts")
            HnTs = wpool.tile([P, 2 * SLAB], F32R, tag="HnTs")
            ATs = wpool.tile([P, 2 * SLAB], F32, tag="ATs")
            CTS = wpool.tile([P, NT * N], F16, tag="CTS")
            BT = wpool.tile([P, NT * 16], F32, tag="BT")

            def compute_body():
                # ---------- encoder (transposed, f32r) ----------
                with tc.tile_pool(name="eps", bufs=4, space="PSUM") as pps:
                    # H1T = relu(W1^T @ E^T + b1)
                    for mh in range(2):
                        for jt in range(2):
                            ps = pps.tile([P, 512], F32, tag="eps")
                            for kd in range(4):
                                nc.tensor.matmul(
                                    ps[:],
                                    W1s[:, kd * HID + mh * P: kd * HID + (mh + 1) * P],
                                    ET[:, kd * N + jt * 512: kd * N + (jt + 1) * 512],
                                    start=(kd == 0), stop=(kd == 3))
                            dstv = H1T[:, mh * N + jt * 512: mh * N + (jt + 1) * 512]
                            if jt == 0:
                                nc.scalar.activation(dstv, ps[:], AF.Relu,
                                                     bias=b1c[:, mh:mh + 1])
                            else:
                                nc.vector.tensor_scalar(dstv, ps[:], b1c[:, mh:mh + 1],
                                                        0.0, ALU.add, ALU.max)
                    # HnT = relu(W2^T @ H1T + b2)
                    for mh in range(2):
                        for jt in range(2):
                            ps = pps.tile([P, 512], F32, tag="eps")
                            for kh in range(2):
                                nc.tensor.matmul(
                                    ps[:],
                                    W2s[:, kh * HID + mh * P: kh * HID + (mh + 1) * P],
                                    H1T[:, kh * N + jt * 512: kh * N + (jt + 1) * 512],
                                    start=(kh == 0), stop=(kh == 1))
                            dstv = HnT[:, mh * N + jt * 512: mh * N + (jt + 1) * 512]
                            if jt == 0:
                                nc.scalar.activation(dstv, ps[:], AF.Relu,
                                                     bias=b2c[:, mh:mh + 1])
                            else:
                                nc.vector.tensor_scalar(dstv, ps[:], b2c[:, mh:mh + 1],
                                                        0.0, ALU.add, ALU.max)
                    # slab encoder: H1Ts, HnTs, ATs (Nf = 128)
                    for mh in range(2):
                        ps = pps.tile([P, SLAB], F32, tag="sps")
                        for kd in range(4):
                            nc.tensor.matmul(
                                ps[:],
                                W1s[:, kd * HID + mh * P: kd * HID + (mh + 1) * P],
                                ETs[:, kd * SLAB:(kd + 1) * SLAB],
                                start=(kd == 0), stop=(kd == 3))
                        nc.scalar.activation(
                            H1Ts[:, mh * SLAB:(mh + 1) * SLAB],
                            ps[:], AF.Relu, bias=b1c[:, mh:mh + 1])
                    for mh in range(2):
                        ps = pps.tile([P, SLAB], F32, tag="sps")
                        for kh in range(2):
                            nc.tensor.matmul(
                                ps[:],
                                W2s[:, kh * HID + mh * P: kh * HID + (mh + 1) * P],
                                H1Ts[:, kh * SLAB:(kh + 1) * SLAB],
                                start=(kh == 0), stop=(kh == 1))
                        nc.scalar.activation(
                            HnTs[:, mh * SLAB:(mh + 1) * SLAB],
                            ps[:], AF.Relu, bias=b2c[:, mh:mh + 1])
                    for mh in range(2):
                        ps = pps.tile([P, SLAB], F32, tag="sps")
                        for kh in range(2):
                            nc.tensor.matmul(
                                ps[:],
                                Was[:, kh * HID + mh * P: kh * HID + (mh + 1) * P],
                                HnTs[:, kh * SLAB:(kh + 1) * SLAB],
                                start=(kh == 0), stop=(kh == 1))
                        nc.scalar.activation(
                            ATs[:, mh * SLAB:(mh + 1) * SLAB],
                            ps[:], AF.Identity, bias=bp1c[:, mh:mh + 1])

                # ---------- BT via DRAM bounce ----------
                for mh in range(2):
                    nc.sync.dma_start(ATd[mh * P:(mh + 1) * P, :],
                                      ATs[:, mh * SLAB:(mh + 1) * SLAB])
                # group c = r*4+s owns i's {g*16+c}; host un-permutes rows
                atd_v = ATd[:].rearrange("(t u) (gg c) -> gg u t c",
                                           u=HSUB, gg=G)
                for g in range(G):
                    dst = BT[g * HSUB:(g + 1) * HSUB, :] \
                        .rearrange("u (t c) -> u t c", c=16)
                    nc.sync.dma_start(dst, atd_v[g])

                # ---------- CTS via replicated-Wb matmuls ----------
                with tc.tile_pool(name="cps", bufs=4, space="PSUM") as cpps:
                    for t in range(NT):
                        for jt in range(2):
                            ps = cpps.tile([P, 512], F32, tag="cps")
                            for kh in range(2):
                                nc.tensor.matmul(
                                    ps[:],
                                    Wbrep[:, (t * 2 + kh) * P:(t * 2 + kh + 1) * P],
                                    HnT[:, kh * N + jt * 512: kh * N + (jt + 1) * 512],
                                    start=(kh == 0), stop=(kh == 1))
                            dst = CTS[:, t * N + jt * 512: t * N + (jt + 1) * 512]
                            if (t * 2 + jt) % 2 == 0:
                                nc.scalar.copy(dst, ps[:])
                            else:
                                nc.vector.tensor_copy(dst, ps[:])

                # ---------- pairwise main loop ----------
                with tc.tile_pool(name="mps", bufs=1, space="PSUM") as mpool:
                    PS = mpool.tile([P, NR * N], F32, tag="PS")
                    ESB = epool.tile([P, N], F32, tag="ESB")
                    OUTSB = wpool.tile([P, N], F32, tag="OUTSB")
                    OUT2 = wpool.tile([P, N], F32, tag="OUT2")
                    for r in range(NR):
                        for t in range(NT):
                            for s in range(NS):
                                R = rpool.tile([P, N], F16, tag="R")
                                bias_col = BT[:, t * 16 + r * 4 + s: t * 16 + r * 4 + s + 1]
                                src = CTS[:, t * N:(t + 1) * N]
                                rel_idx = (r * NT + t) * NS + s
                                if rel_idx % 6 == 0:
                                    nc.scalar.activation(R[:], src, AF.Relu,
                                                         bias=bias_col)
                                else:
                                    nc.vector.tensor_scalar(R[:], src, bias_col,
                                                            0.0, ALU.add, ALU.max)
                                for jt in range(NJT):
                                    nc.tensor.matmul(
                                        PS[32 * s:32 * s + G,
                                           r * N + jt * 512: r * N + (jt + 1) * 512],
                                        Wst[:, t * G:(t + 1) * G],
                                        R[:, jt * 512:(jt + 1) * 512],
                                        start=(t == 0), stop=(t == NT - 1),
                                        tile_position=(0, 32 * s))
                        # drain round r: exp(logits + bp2) from PSUM, compact via DMA
                        for s in range(NS):
                            nc.scalar.activation(
                                ESB[32 * s:32 * s + G, :],
                                PS[32 * s:32 * s + G, r * N:(r + 1) * N],
                                AF.Exp, bias=bp2c[32 * s:32 * s + G, 0:1])
                            nc.sync.dma_start(
                                OUTSB[r * 32 + s * G: r * 32 + (s + 1) * G, :],
                                ESB[32 * s:32 * s + G, :])
                    # softplus = ln(1 + exp(x)); diagonal fixed on host
                    OUT2 = wpool.tile([P, N], F32, tag="OUT2")
                    nc.scalar.activation(OUT2[:], OUTSB[:], AF.Ln, bias=1.0)
                    nc.sync.dma_start(dY.ap(), OUT2[:])

            if repeat == 1:
                compute_body()
            else:
                with tc.For_i(0, repeat, 1):
                    compute_body()

    nc.compile()
    return nc


def _prep_inputs(E, W1, b1, W2, b2, Wp1, bp1, Wp2, bp2):
    f32 = np.float32
    E = np.asarray(E, f32)
    W1 = np.asarray(W1, f32)
    b1 = np.asarray(b1, f32)
    W2 = np.asarray(W2, f32)
    b2 = np.asarray(b2, f32)
    Wp1 = np.asarray(Wp1, f32)
    bp1 = np.asarray(bp1, f32)
    Wp2 = np.asarray(Wp2, f32)
    bp2 = np.asarray(bp2, f32)

    ET = np.ascontiguousarray(E.T)                      # (512, 1024)
    W1s = np.ascontiguousarray(
        W1.reshape(4, P, HID).transpose(1, 0, 2).reshape(P, 4 * HID))
    W2s = np.ascontiguousarray(
        W2.reshape(2, P, HID).transpose(1, 0, 2).reshape(P, 2 * HID))
    Wa, Wb = Wp1[:HID], Wp1[HID:]
    Was = np.ascontiguousarray(
        Wa.reshape(2, P, HID).transpose(1, 0, 2).reshape(P, 2 * HID))
    Wbrep = np.zeros((NT, 2, P, P), np.float32)
    for t in range(NT):
        for kh in range(2):
            Wbrep[t, kh] = np.tile(Wb[kh * P:(kh + 1) * P, t * HSUB:(t + 1) * HSUB],
                                   (1, G))
    b1c = np.ascontiguousarray(b1.reshape(2, P).T)
    b2c = np.ascontiguousarray(b2.reshape(2, P).T)
    bp1c = np.ascontiguousarray(bp1.reshape(2, P).T)

    Wst = np.zeros((P, NT * G), np.float16)
    w = Wp2[:, 0]
    for t in range(NT):
        for g in range(G):
            for u in range(HSUB):
                Wst[g * HSUB + u, t * G + g] = w[t * HSUB + u]

    bp2c = np.full((P, 1), bp2[0], np.float32)
    common = {
        "ET": ET, "W1s": W1s, "W2s": W2s, "Was": Was, "Wbrep": Wbrep,
        "b1c": b1c, "b2c": b2c, "bp1c": bp1c, "bp2c": bp2c, "Wst": Wst,
    }
    in_maps = []
    for k in range(NCORE):
        m = dict(common)
        m["ETs"] = np.ascontiguousarray(E[k * SLAB:(k + 1) * SLAB, :].T)
        in_maps.append(m)
    return in_maps, float(bp2[0])


def kernel(E, W1, b1, W2, b2, Wp1, bp1, Wp2, bp2):
    from concourse.bass_utils import run_bass_kernel_spmd

    if "nc" not in _CACHE:
        _CACHE["nc"] = _build_program()
    nc = _CACHE["nc"]

    in_maps, _ = _prep_inputs(E, W1, b1, W2, b2, Wp1, bp1, Wp2, bp2)
    res = run_bass_kernel_spmd(nc, in_maps, list(range(NCORE)))
    # device writes row p = c*8+g for slab-local i = g*16+c; un-permute
    slabs = [res.results[k]["Y"].reshape(16, 8, N).transpose(1, 0, 2).reshape(SLAB, N)
             for k in range(NCORE)]
    out = np.concatenate(slabs, axis=0)
    np.fill_diagonal(out, 0.0)
    return np.ascontiguousarray(out.astype(np.float32))

